# revision 10
# baseline (speedup 1.0000x reference)
# Trainium2 Bass kernel for nn_Invo2D, v5 (p-major store, host unpermute): 2D-tiled partitions, padded-x
# single load, bf16 output (host upcast).
#
#   t2 = x @ Wc + bc     (Wc = W1@W2, bc = b1@W2+b2 host-folded, bf16)
#   out[pix, co] = sum_j t2[pix, 9*(co//16)+j] * x_tap(f)[pix, ch(f)], f = 9*co+j
#
# Per core (1 image): partition p = hb*8 + wb (16 h-blocks x 8 w-blocks);
# each partition owns a 4x8 pixel block with a 1-pixel 2D halo: free dim =
# (6 h-slots x 10 w-slots x 256 ch).  x is zero-padded to [66,66,256] bf16
# on the host so the halo loads are a single in-bounds DMA per h-slice.
# Both spatial taps are free-dim offsets -> ONE x buffer, no shifted copies.
# Products: DVE 2x piece-muls into 9-packed M9 (2 rotating hl-chunk bufs).
# Fold over 9 taps: TensorE identity-lhsT PSUM-accumulate; Act evacuates
# PSUM -> SBUF bf16; DMA stores bf16; host upcasts to f32.
import numpy as np
import ml_dtypes

H, W, C = 64, 64, 256
M144 = 144
NCORES = 8
HS, WS = 6, 10          # h-slots, w-slots per partition (4+2, 8+2)
XF = HS * WS * C        # 15360 bf16 per partition
WLC = 8                 # positions per chunk = one hl row
NCHUNK = 4
M9F = WLC * 2304
W9F = WLC * M144
PW = 66                 # padded row length

_cache = {}


def _rect_decomp(r0, r1):
    out = []
    gc0, j0 = divmod(r0, 9)
    if j0 != 0:
        end = min(r1, (gc0 + 1) * 9)
        out.append((gc0, 1, j0, end - r0))
        r0 = end
        if r0 == r1:
            return out
        gc0, j0 = divmod(r0, 9)
    nfull = (r1 - r0) // 9
    if nfull:
        out.append((gc0, nfull, 0, 9))
        r0 += nfull * 9
        gc0 += nfull
    if r0 < r1:
        out.append((gc0, 1, 0, r1 - r0))
    return out


def _build_pieces():
    pieces = []
    for g in range(16):
        f_lo, f_hi = 144 * g, 144 * g + 144
        cuts = [f_lo] + [256 * k for k in range(1, 9) if f_lo < 256 * k < f_hi] + [f_hi]
        for a, b in zip(cuts, cuts[1:]):
            ki = a // 256
            for (gc0, ngc, j0, nj) in _rect_decomp(a - 144 * g, b - 144 * g):
                pieces.append((g, gc0, ngc, j0, nj, ki))
    return pieces


def _build_program(n_repeat=1, with_bias=True):
    import concourse.bass as bass
    import concourse.tile as tile
    from concourse import bacc, mybir
    from concourse.masks import make_identity

    f32 = mybir.dt.float32
    bf16 = mybir.dt.bfloat16
    AP = bass.AP

    nc = bacc.Bacc(None, target_bir_lowering=False)
    x_d = nc.dram_tensor("xpad", [PW, PW, C], bf16, kind="ExternalInput")
    wc_d = nc.dram_tensor("wc", [2, 128, M144], bf16, kind="ExternalInput")
    bc_d = nc.dram_tensor("bc", [1, M144], bf16, kind="ExternalInput")
    out_d = nc.dram_tensor("out", [128, 32 * C], bf16, kind="ExternalOutput")

    PIECES = _build_pieces()

    with tile.TileContext(nc) as tc:
        with (
            tc.tile_pool(name="singles", bufs=1) as singles,
            tc.tile_pool(name="xbufs", bufs=1) as xbufs,
            tc.tile_pool(name="w9p", bufs=1) as w9p,
            tc.tile_pool(name="m9p", bufs=2) as m9p,
            tc.tile_pool(name="outp", bufs=2) as outp,
            tc.tile_pool(name="pst", bufs=2, space="PSUM") as pst,
            tc.tile_pool(name="ps2p", bufs=3, space="PSUM") as ps2p,
            tc.tile_pool(name="foldp", bufs=3, space="PSUM") as foldp,
        ):
            ident = singles.tile([128, 128], bf16)
            make_identity(nc, ident[:])
            wc0 = singles.tile([128, M144], bf16)
            wc1 = singles.tile([128, M144], bf16)
            nc.sync.dma_start(out=wc0[:], in_=wc_d[0])
            nc.sync.dma_start(out=wc1[:], in_=wc_d[1])
            bcsb = singles.tile([1, M144], bf16)
            nc.sync.dma_start(out=bcsb[:], in_=bc_d[:])
            ones1 = singles.tile([1, 128], bf16)
            nc.vector.memset(ones1[:], 1.0)
            xcm = singles.tile([128, 2 * 32 * 128], bf16)
            W9c = [w9p.tile([128, W9F], bf16, name=f"w9_{c}", tag=f"w9_{c}")
                   for c in range(NCHUNK)]

            for _rep in range(n_repeat):
                _image_body(nc, bass, mybir,
                            xbufs, m9p, outp, pst, ps2p, foldp,
                            ident, wc0, wc1, bcsb, ones1, xcm, W9c,
                            x_d, out_d, PIECES, with_bias)
    nc.compile()
    return nc


def _image_body(nc, bass, mybir,
                xbufs, m9p, outp, pst, ps2p, foldp,
                ident, wc0, wc1, bcsb, ones1, xcm, W9c,
                x_d, out_d, PIECES, with_bias):
    f32 = mybir.dt.float32
    bf16 = mybir.dt.bfloat16
    AP = bass.AP

    X = xbufs.tile([128, XF], bf16, tag="X")
    # h-slice loads: slot row s (0..5) <- xpad row hb*4 + s, cols wb*8..+10.
    # partition p = hb*8 + wb.  Order: s=1 gates grp0's transposes, then s=2
    # (grp1), s=0 (chunk0 products), then the rest.
    for s in (1, 2, 0, 3, 4, 5):
        nc.sync.dma_start(
            out=AP(X.tensor, s * WS * C, [[XF, 128], [1, WS * C]]),
            in_=AP(x_d, s * PW * C,
                   [[4 * PW * C, 16], [8 * C, 8], [1, WS * C]]),
        )

    # per hl-row grp: transposes -> xcm, t2 matmuls, W9 scatter.
    # grp0 runs at 4-tile granularity to shorten the path to the first
    # products; later grps use full-8 batches.
    for grp in range(4):
        nsub = 2 if grp == 0 else 1
        for sub in range(nsub):
            tts = range(sub * 8 // nsub, (sub + 1) * 8 // nsub)
            for half in range(2):
                pt = pst.tile([128, 1024 // nsub], bf16, padded_shape=[128, 1024])
                for i, tt in enumerate(tts):
                    slot = (grp + 1) * WS + (tt + 1)
                    nc.tensor.transpose(
                        out=pt[:, i * 128:(i + 1) * 128],
                        in_=AP(X.tensor, slot * C + half * 128,
                               [[XF, 128], [1, 128]]),
                        identity=ident[:],
                    )
                cpeng = nc.vector.tensor_copy if grp < 2 else (
                    lambda out, in_: nc.scalar.copy(out=out, in_=in_))
                cpeng(
                    out=xcm[:, (half * 32 + grp * 8 + tts[0]) * 128:
                            (half * 32 + grp * 8 + tts[-1] + 1) * 128],
                    in_=pt[:],
                )
            for tt in tts:
                t = grp * 8 + tt
                ps2 = ps2p.tile([128, M144], f32)
                xc0 = AP(xcm.tensor, t * 128, [[2 * 32 * 128, 128], [1, 128]])
                xc1 = AP(xcm.tensor, (32 + t) * 128, [[2 * 32 * 128, 128], [1, 128]])
                nc.tensor.matmul(ps2[:], lhsT=xc0, rhs=wc0[:], start=True, stop=False)
                nc.tensor.matmul(ps2[:], lhsT=xc1, rhs=wc1[:], start=False,
                                 stop=not with_bias)
                if with_bias:
                    nc.tensor.matmul(ps2[:], lhsT=ones1[:], rhs=bcsb[:],
                                     start=False, stop=True)
                nc.scalar.copy(
                    out=AP(W9c[grp].tensor, tt * M144, [[W9F, 128], [1, M144]]),
                    in_=ps2[:],
                )

    # products (DVE) -> M9; fold (PE) -> PSUM; evacuate (Act); store
    NBLK = (WLC * 256) // 512
    for c in range(NCHUNK):
        M9 = m9p.tile([128, M9F], bf16, tag="m9")
        wl_splits = [(0, WLC)] if c < NCHUNK - 1 else [(0, 6), (6, WLC)]
        for (wa, wb) in wl_splits:
            for (g, gc0, ngc, j0, nj, ki) in PIECES:
                di, dj = ki // 3 - 1, ki % 3 - 1
                f0 = 144 * g + 9 * gc0 + j0
                in0 = AP(X.tensor,
                         ((c + di + 1) * WS + wa + dj + 1) * C + f0 - 256 * ki,
                         [[XF, 128], [256, wb - wa], [9, ngc], [1, nj]])
                in1 = AP(W9c[c].tensor, wa * M144 + 9 * g + j0,
                         [[W9F, 128], [M144, wb - wa], [0, ngc], [1, nj]])
                o = AP(M9.tensor, wa * 2304 + f0,
                       [[M9F, 128], [2304, wb - wa], [9, ngc], [1, nj]])
                nc.vector.tensor_mul(o, in0, in1)
        outc = outp.tile([128, WLC * 256], bf16, tag="outc")
        last = c == NCHUNK - 1
        for b in range(NBLK):
            ps = foldp.tile([128, 512], f32)
            for j in range(9):
                nc.tensor.matmul(
                    ps[:],
                    lhsT=ident[:],
                    rhs=AP(M9.tensor, b * 2 * 2304 + j,
                           [[M9F, 128], [2304, 2], [9, 256]]),
                    start=(j == 0),
                    stop=(j == 8),
                )
            if last and b == NBLK - 1:
                nc.vector.tensor_copy(out=outc[:, b * 512:(b + 1) * 512], in_=ps[:])
            else:
                nc.scalar.copy(out=outc[:, b * 512:(b + 1) * 512], in_=ps[:])
        # store hl-row c in partition-major layout; host un-permutes.
        # Last chunk: store first 3 blocks early, tail block separately.
        if last:
            nc.sync.dma_start(
                out=AP(out_d, c * WLC * 256, [[32 * C, 128], [1, 3 * 512]]),
                in_=AP(outc.tensor, 0, [[WLC * 256, 128], [1, 3 * 512]]),
            )
            nc.sync.dma_start(
                out=AP(out_d, c * WLC * 256 + 3 * 512, [[32 * C, 128], [1, 512]]),
                in_=AP(outc.tensor, 3 * 512, [[WLC * 256, 128], [1, 512]]),
            )
        else:
            nc.sync.dma_start(
                out=AP(out_d, c * WLC * 256, [[32 * C, 128], [1, WLC * 256]]),
                in_=AP(outc.tensor, 0, [[WLC * 256, 128], [1, WLC * 256]]),
            )


def _get_program(n_repeat=1, with_bias=True):
    key = ("nc", n_repeat, with_bias)
    if key not in _cache:
        _cache[key] = _build_program(n_repeat, with_bias)
    return _cache[key]


def _make_in_maps(inputs):
    x, W1, b1, W2, b2 = (inputs[k] for k in ("x", "W1", "b1", "W2", "b2"))
    bf = ml_dtypes.bfloat16
    Wc = (np.asarray(W1, np.float32) @ np.asarray(W2, np.float32))
    bc = (np.asarray(b1, np.float32) @ np.asarray(W2, np.float32)
          + np.asarray(b2, np.float32))
    wc_h = np.ascontiguousarray(Wc.astype(bf).reshape(2, 128, M144))
    bc_h = np.ascontiguousarray(bc.astype(bf).reshape(1, M144))
    xp = np.zeros((NCORES, PW, PW, C), dtype=bf)
    xp[:, 1:65, 1:65, :] = np.asarray(x).astype(bf)
    return [
        {
            "xpad": np.ascontiguousarray(xp[i]),
            "wc": wc_h,
            "bc": bc_h,
        }
        for i in range(NCORES)
    ]


def _unpermute(raw):
    # [128, 32*256] p-major -> [64, 64, 256]: p = hb*8+wb, free = (hl, wl, co)
    return (np.asarray(raw).reshape(16, 8, 4, 8, 256)
            .transpose(0, 2, 1, 3, 4)
            .reshape(64, 64, 256))


def kernel(x, W1, b1, W2, b2, trace=False):
    from concourse.bass_utils import run_bass_kernel_spmd

    bc = np.asarray(b1, np.float32) @ np.asarray(W2, np.float32) + np.asarray(
        b2, np.float32)
    nc = _get_program(with_bias=bool(np.any(bc != 0.0)))
    in_maps = _make_in_maps(dict(x=x, W1=W1, b1=b1, W2=W2, b2=b2))
    res = run_bass_kernel_spmd(nc, in_maps, core_ids=list(range(NCORES)),
                               trace=trace)
    out = np.stack([_unpermute(res.results[i]["out"]) for i in range(NCORES)],
                   axis=0).astype(np.float32)
    if trace:
        return out, res
    return out


# revision 12
# speedup vs baseline: 1.3639x; 1.3639x over previous
# Trainium2 Bass kernel for nn_Invo2D, v5 (p-major store, host unpermute): 2D-tiled partitions, padded-x
# single load, bf16 output (host upcast).
#
#   t2 = x @ Wc + bc     (Wc = W1@W2, bc = b1@W2+b2 host-folded, bf16)
#   out[pix, co] = sum_j t2[pix, 9*(co//16)+j] * x_tap(f)[pix, ch(f)], f = 9*co+j
#
# Per core (1 image): partition p = hb*8 + wb (16 h-blocks x 8 w-blocks);
# each partition owns a 4x8 pixel block with a 1-pixel 2D halo: free dim =
# (6 h-slots x 10 w-slots x 256 ch).  x is zero-padded to [66,66,256] bf16
# on the host so the halo loads are a single in-bounds DMA per h-slice.
# Both spatial taps are free-dim offsets -> ONE x buffer, no shifted copies.
# Products: DVE 2x piece-muls into 9-packed M9 (2 rotating hl-chunk bufs).
# Fold over 9 taps: TensorE identity-lhsT PSUM-accumulate; Act evacuates
# PSUM -> SBUF bf16; DMA stores bf16; host upcasts to f32.
import numpy as np
import ml_dtypes

H, W, C = 64, 64, 256
M144 = 144
NCORES = 8
HS, WS = 6, 10          # h-slots, w-slots per partition (4+2, 8+2)
XF = HS * WS * C        # 15360 bf16 per partition
WLC = 8                 # positions per chunk = one hl row
NCHUNK = 4
M9F = WLC * 2304
W9F = WLC * M144
PW = 66                 # padded row length

_cache = {}


def _rect_decomp(r0, r1):
    out = []
    gc0, j0 = divmod(r0, 9)
    if j0 != 0:
        end = min(r1, (gc0 + 1) * 9)
        out.append((gc0, 1, j0, end - r0))
        r0 = end
        if r0 == r1:
            return out
        gc0, j0 = divmod(r0, 9)
    nfull = (r1 - r0) // 9
    if nfull:
        out.append((gc0, nfull, 0, 9))
        r0 += nfull * 9
        gc0 += nfull
    if r0 < r1:
        out.append((gc0, 1, 0, r1 - r0))
    return out


def _build_pieces():
    pieces = []
    for g in range(16):
        f_lo, f_hi = 144 * g, 144 * g + 144
        cuts = [f_lo] + [256 * k for k in range(1, 9) if f_lo < 256 * k < f_hi] + [f_hi]
        for a, b in zip(cuts, cuts[1:]):
            ki = a // 256
            for (gc0, ngc, j0, nj) in _rect_decomp(a - 144 * g, b - 144 * g):
                pieces.append((g, gc0, ngc, j0, nj, ki))
    return pieces


def _build_program(n_repeat=1, with_bias=True, pool_kis=()):
    import concourse.bass as bass
    import concourse.tile as tile
    from concourse import bacc, mybir
    from concourse.masks import make_identity

    f32 = mybir.dt.float32
    bf16 = mybir.dt.bfloat16
    AP = bass.AP

    nc = bacc.Bacc(None, target_bir_lowering=False)
    x_d = nc.dram_tensor("xpad", [PW, PW, C], bf16, kind="ExternalInput")
    wc_d = nc.dram_tensor("wc", [2, 128, M144], bf16, kind="ExternalInput")
    bc_d = nc.dram_tensor("bc", [1, M144], bf16, kind="ExternalInput")
    out_d = nc.dram_tensor("out", [128, 32 * C], bf16, kind="ExternalOutput")

    PIECES = _build_pieces()

    with tile.TileContext(nc) as tc:
        with (
            tc.tile_pool(name="singles", bufs=1) as singles,
            tc.tile_pool(name="xbufs", bufs=1) as xbufs,
            tc.tile_pool(name="w9p", bufs=1) as w9p,
            tc.tile_pool(name="m9p", bufs=2) as m9p,
            tc.tile_pool(name="outp", bufs=2) as outp,
            tc.tile_pool(name="pst", bufs=2, space="PSUM") as pst,
            tc.tile_pool(name="ps2p", bufs=3, space="PSUM") as ps2p,
            tc.tile_pool(name="foldp", bufs=3, space="PSUM") as foldp,
        ):
            ident = singles.tile([128, 128], bf16)
            make_identity(nc, ident[:])
            wc0 = singles.tile([128, M144], bf16)
            wc1 = singles.tile([128, M144], bf16)
            nc.scalar.dma_start(out=wc0[:], in_=wc_d[0])
            nc.scalar.dma_start(out=wc1[:], in_=wc_d[1])
            bcsb = singles.tile([1, M144], bf16)
            ones1 = singles.tile([1, 128], bf16)
            if with_bias:
                nc.scalar.dma_start(out=bcsb[:], in_=bc_d[:])
                nc.vector.memset(ones1[:], 1.0)
            xcm = singles.tile([128, 2 * 32 * 128], bf16)
            W9c = [w9p.tile([128, W9F], bf16, name=f"w9_{c}", tag=f"w9_{c}")
                   for c in range(NCHUNK)]

            for _rep in range(n_repeat):
                _image_body(nc, bass, mybir,
                            xbufs, m9p, outp, pst, ps2p, foldp,
                            ident, wc0, wc1, bcsb, ones1, xcm, W9c,
                            x_d, out_d, PIECES, with_bias, pool_kis)
    nc.compile()
    return nc


def _image_body(nc, bass, mybir,
                xbufs, m9p, outp, pst, ps2p, foldp,
                ident, wc0, wc1, bcsb, ones1, xcm, W9c,
                x_d, out_d, PIECES, with_bias, pool_kis=()):
    f32 = mybir.dt.float32
    bf16 = mybir.dt.bfloat16
    AP = bass.AP

    X = xbufs.tile([128, XF], bf16, tag="X")
    # h-slice loads: slot row s (0..5) <- xpad row hb*4 + s, cols wb*8..+10.
    # partition p = hb*8 + wb.  Order: s=1 gates grp0's transposes, then s=2
    # (grp1), s=0 (chunk0 products), then the rest.
    nc.sync.dma_start(
        out=AP(X.tensor, WS * C, [[XF, 128], [1, 5 * C]]),
        in_=AP(x_d, PW * C, [[4 * PW * C, 16], [8 * C, 8], [1, 5 * C]]),
    )
    nc.sync.dma_start(
        out=AP(X.tensor, (WS + 5) * C, [[XF, 128], [1, 5 * C]]),
        in_=AP(x_d, (PW + 5) * C, [[4 * PW * C, 16], [8 * C, 8], [1, 5 * C]]),
    )
    for s in (2, 0, 3, 4, 5):
        nc.sync.dma_start(
            out=AP(X.tensor, s * WS * C, [[XF, 128], [1, WS * C]]),
            in_=AP(x_d, s * PW * C,
                   [[4 * PW * C, 16], [8 * C, 8], [1, WS * C]]),
        )

    # per hl-row grp: transposes -> xcm, t2 matmuls, W9 scatter.
    # grp0 runs at 4-tile granularity to shorten the path to the first
    # products; later grps use full-8 batches.
    for grp in range(4):
        nsub = 2 if grp == 0 else 1
        for sub in range(nsub):
            tts = range(sub * 8 // nsub, (sub + 1) * 8 // nsub)
            for half in range(2):
                pt = pst.tile([128, 1024 // nsub], bf16, padded_shape=[128, 1024])
                for i, tt in enumerate(tts):
                    slot = (grp + 1) * WS + (tt + 1)
                    nc.tensor.transpose(
                        out=pt[:, i * 128:(i + 1) * 128],
                        in_=AP(X.tensor, slot * C + half * 128,
                               [[XF, 128], [1, 128]]),
                        identity=ident[:],
                    )
                cpeng = nc.vector.tensor_copy if grp < 2 else (
                    lambda out, in_: nc.scalar.copy(out=out, in_=in_))
                cpeng(
                    out=xcm[:, (half * 32 + grp * 8 + tts[0]) * 128:
                            (half * 32 + grp * 8 + tts[-1] + 1) * 128],
                    in_=pt[:],
                )
            for tt in tts:
                t = grp * 8 + tt
                ps2 = ps2p.tile([128, M144], f32)
                xc0 = AP(xcm.tensor, t * 128, [[2 * 32 * 128, 128], [1, 128]])
                xc1 = AP(xcm.tensor, (32 + t) * 128, [[2 * 32 * 128, 128], [1, 128]])
                nc.tensor.matmul(ps2[:], lhsT=xc0, rhs=wc0[:], start=True, stop=False)
                nc.tensor.matmul(ps2[:], lhsT=xc1, rhs=wc1[:], start=False,
                                 stop=not with_bias)
                if with_bias:
                    nc.tensor.matmul(ps2[:], lhsT=ones1[:], rhs=bcsb[:],
                                     start=False, stop=True)
                nc.scalar.copy(
                    out=AP(W9c[grp].tensor, tt * M144, [[W9F, 128], [1, M144]]),
                    in_=ps2[:],
                )

    # products (DVE+Pool, g-ascending) -> M9; fold (PE) per g-quarter;
    # evacuate (Act); store.  Groups are 9-strided in f (f = 9*(16g+gc)),
    # so a fold block = (all 8 wl) x (64 groups) only needs pieces for
    # g in [4q, 4q+4) -- the fold trails the g-ordered products closely.
    for c in range(NCHUNK):
        M9 = m9p.tile([128, M9F], bf16, tag="m9")
        for (g, gc0, ngc, j0, nj, ki) in PIECES:
            di, dj = ki // 3 - 1, ki % 3 - 1
            f0 = 144 * g + 9 * gc0 + j0
            in0 = AP(X.tensor,
                     ((c + di + 1) * WS + dj + 1) * C + f0 - 256 * ki,
                     [[XF, 128], [256, WLC], [9, ngc], [1, nj]])
            in1 = AP(W9c[c].tensor, 9 * g + j0,
                     [[W9F, 128], [M144, WLC], [0, ngc], [1, nj]])
            o = AP(M9.tensor, f0,
                   [[M9F, 128], [2304, WLC], [9, ngc], [1, nj]])
            kis = pool_kis if c < NCHUNK - 1 else (1, 4, 7)
            eng = nc.gpsimd if ki in kis else nc.vector
            eng.tensor_mul(o, in0, in1)
        outc = outp.tile([128, WLC * 256], bf16, tag="outc")
        last = c == NCHUNK - 1
        for q in range(4):
            ps = foldp.tile([128, 512], f32)
            for j in range(9):
                nc.tensor.matmul(
                    ps[:],
                    lhsT=ident[:],
                    rhs=AP(M9.tensor, 576 * q + j,
                           [[M9F, 128], [2304, WLC], [9, 64]]),
                    start=(j == 0),
                    stop=(j == 8),
                )
            odst = AP(outc.tensor, 64 * q,
                      [[WLC * 256, 128], [256, WLC], [1, 64]])
            if last and q == 3:
                nc.vector.tensor_copy(out=odst, in_=ps[:])
            else:
                nc.scalar.copy(out=odst, in_=ps[:])
        # store hl-row c in partition-major layout; host un-permutes.
        # Last chunk: store first 3 blocks early, tail block separately.
        if last:
            nc.sync.dma_start(
                out=AP(out_d, c * WLC * 256, [[32 * C, 128], [1, 3 * 512]]),
                in_=AP(outc.tensor, 0, [[WLC * 256, 128], [1, 3 * 512]]),
            )
            nc.sync.dma_start(
                out=AP(out_d, c * WLC * 256 + 3 * 512, [[32 * C, 128], [1, 512]]),
                in_=AP(outc.tensor, 3 * 512, [[WLC * 256, 128], [1, 512]]),
            )
        else:
            nc.sync.dma_start(
                out=AP(out_d, c * WLC * 256, [[32 * C, 128], [1, WLC * 256]]),
                in_=AP(outc.tensor, 0, [[WLC * 256, 128], [1, WLC * 256]]),
            )


def _get_program(n_repeat=1, with_bias=True, pool_kis=(1, 7)):
    key = ("nc", n_repeat, with_bias, tuple(pool_kis))
    if key not in _cache:
        _cache[key] = _build_program(n_repeat, with_bias, pool_kis)
    return _cache[key]


def _make_in_maps(inputs):
    x, W1, b1, W2, b2 = (inputs[k] for k in ("x", "W1", "b1", "W2", "b2"))
    bf = ml_dtypes.bfloat16
    Wc = (np.asarray(W1, np.float32) @ np.asarray(W2, np.float32))
    bc = (np.asarray(b1, np.float32) @ np.asarray(W2, np.float32)
          + np.asarray(b2, np.float32))
    wc_h = np.ascontiguousarray(Wc.astype(bf).reshape(2, 128, M144))
    bc_h = np.ascontiguousarray(bc.astype(bf).reshape(1, M144))
    xp = np.zeros((NCORES, PW, PW, C), dtype=bf)
    xp[:, 1:65, 1:65, :] = np.asarray(x).astype(bf)
    return [
        {
            "xpad": np.ascontiguousarray(xp[i]),
            "wc": wc_h,
            "bc": bc_h,
        }
        for i in range(NCORES)
    ]


def _unpermute(raw):
    # [128, 32*256] p-major -> [64, 64, 256]: p = hb*8+wb, free = (hl, wl, co)
    return (np.asarray(raw).reshape(16, 8, 4, 8, 256)
            .transpose(0, 2, 1, 3, 4)
            .reshape(64, 64, 256))


def kernel(x, W1, b1, W2, b2, trace=False):
    from concourse.bass_utils import run_bass_kernel_spmd

    bc = np.asarray(b1, np.float32) @ np.asarray(W2, np.float32) + np.asarray(
        b2, np.float32)
    nc = _get_program(with_bias=bool(np.any(bc != 0.0)))
    in_maps = _make_in_maps(dict(x=x, W1=W1, b1=b1, W2=W2, b2=b2))
    res = run_bass_kernel_spmd(nc, in_maps, core_ids=list(range(NCORES)),
                               trace=trace)
    out = np.stack([_unpermute(res.results[i]["out"]) for i in range(NCORES)],
                   axis=0).astype(np.float32)
    if trace:
        return out, res
    return out


# revision 13
# speedup vs baseline: 1.4245x; 1.0444x over previous
# Trainium2 Bass kernel for nn_Invo2D, v5 (p-major store, host unpermute): 2D-tiled partitions, padded-x
# single load, bf16 output (host upcast).
#
#   t2 = x @ Wc + bc     (Wc = W1@W2, bc = b1@W2+b2 host-folded, bf16)
#   out[pix, co] = sum_j t2[pix, 9*(co//16)+j] * x_tap(f)[pix, ch(f)], f = 9*co+j
#
# Per core (1 image): partition p = hb*8 + wb (16 h-blocks x 8 w-blocks);
# each partition owns a 4x8 pixel block with a 1-pixel 2D halo: free dim =
# (6 h-slots x 10 w-slots x 256 ch).  x is zero-padded to [66,66,256] bf16
# on the host so the halo loads are a single in-bounds DMA per h-slice.
# Both spatial taps are free-dim offsets -> ONE x buffer, no shifted copies.
# Products: DVE 2x piece-muls into 9-packed M9 (2 rotating hl-chunk bufs).
# Fold over 9 taps: TensorE identity-lhsT PSUM-accumulate; Act evacuates
# PSUM -> SBUF bf16; DMA stores bf16; host upcasts to f32.
import numpy as np
import ml_dtypes

H, W, C = 64, 64, 256
M144 = 144
NCORES = 8
HS, WS = 6, 10          # h-slots, w-slots per partition (4+2, 8+2)
XF = HS * WS * C        # 15360 bf16 per partition
WLC = 8                 # positions per chunk = one hl row
NCHUNK = 4
M9F = WLC * 2304
W9F = WLC * M144
PW = 66                 # padded row length

_cache = {}


def _rect_decomp(r0, r1):
    out = []
    gc0, j0 = divmod(r0, 9)
    if j0 != 0:
        end = min(r1, (gc0 + 1) * 9)
        out.append((gc0, 1, j0, end - r0))
        r0 = end
        if r0 == r1:
            return out
        gc0, j0 = divmod(r0, 9)
    nfull = (r1 - r0) // 9
    if nfull:
        out.append((gc0, nfull, 0, 9))
        r0 += nfull * 9
        gc0 += nfull
    if r0 < r1:
        out.append((gc0, 1, 0, r1 - r0))
    return out


def _build_pieces():
    pieces = []
    for g in range(16):
        f_lo, f_hi = 144 * g, 144 * g + 144
        cuts = [f_lo] + [256 * k for k in range(1, 9) if f_lo < 256 * k < f_hi] + [f_hi]
        for a, b in zip(cuts, cuts[1:]):
            ki = a // 256
            for (gc0, ngc, j0, nj) in _rect_decomp(a - 144 * g, b - 144 * g):
                pieces.append((g, gc0, ngc, j0, nj, ki))
    return pieces


def _build_program(n_repeat=1, with_bias=True, pool_kis=()):
    import concourse.bass as bass
    import concourse.tile as tile
    from concourse import bacc, mybir
    from concourse.masks import make_identity

    f32 = mybir.dt.float32
    bf16 = mybir.dt.bfloat16
    AP = bass.AP

    nc = bacc.Bacc(None, target_bir_lowering=False)
    x_d = nc.dram_tensor("xpad", [PW, PW, C], bf16, kind="ExternalInput")
    wc_d = nc.dram_tensor("wc", [2, 128, M144], bf16, kind="ExternalInput")
    bc_d = nc.dram_tensor("bc", [1, M144], bf16, kind="ExternalInput")
    out_d = nc.dram_tensor("out", [128, 32 * C], bf16, kind="ExternalOutput")

    PIECES = _build_pieces()

    with tile.TileContext(nc) as tc:
        with (
            tc.tile_pool(name="singles", bufs=1) as singles,
            tc.tile_pool(name="xbufs", bufs=1) as xbufs,
            tc.tile_pool(name="w9p", bufs=1) as w9p,
            tc.tile_pool(name="m9p", bufs=2) as m9p,
            tc.tile_pool(name="outp", bufs=2) as outp,
            tc.tile_pool(name="pst", bufs=2, space="PSUM") as pst,
            tc.tile_pool(name="ps2p", bufs=3, space="PSUM") as ps2p,
            tc.tile_pool(name="foldp", bufs=3, space="PSUM") as foldp,
        ):
            ident = singles.tile([128, 128], bf16)
            make_identity(nc, ident[:])
            wc0 = singles.tile([128, M144], bf16)
            wc1 = singles.tile([128, M144], bf16)
            nc.scalar.dma_start(out=wc0[:], in_=wc_d[0])
            nc.scalar.dma_start(out=wc1[:], in_=wc_d[1])
            bcsb = singles.tile([1, M144], bf16)
            ones1 = singles.tile([1, 128], bf16)
            if with_bias:
                nc.scalar.dma_start(out=bcsb[:], in_=bc_d[:])
                nc.vector.memset(ones1[:], 1.0)
            xcm = singles.tile([128, 2 * 32 * 128], bf16)
            W9c = [w9p.tile([128, W9F], bf16, name=f"w9_{c}", tag=f"w9_{c}")
                   for c in range(NCHUNK)]

            for _rep in range(n_repeat):
                _image_body(nc, bass, mybir,
                            xbufs, m9p, outp, pst, ps2p, foldp,
                            ident, wc0, wc1, bcsb, ones1, xcm, W9c,
                            x_d, out_d, PIECES, with_bias, pool_kis)
    nc.compile()
    return nc


def _image_body(nc, bass, mybir,
                xbufs, m9p, outp, pst, ps2p, foldp,
                ident, wc0, wc1, bcsb, ones1, xcm, W9c,
                x_d, out_d, PIECES, with_bias, pool_kis=()):
    f32 = mybir.dt.float32
    bf16 = mybir.dt.bfloat16
    AP = bass.AP

    X = xbufs.tile([128, XF], bf16, tag="X")
    # h-slice loads: slot row s (0..5) <- xpad row hb*4 + s, cols wb*8..+10.
    # partition p = hb*8 + wb.  Order: s=1 gates grp0's transposes, then s=2
    # (grp1), s=0 (chunk0 products), then the rest.
    nc.sync.dma_start(
        out=AP(X.tensor, WS * C, [[XF, 128], [1, 5 * C]]),
        in_=AP(x_d, PW * C, [[4 * PW * C, 16], [8 * C, 8], [1, 5 * C]]),
    )
    nc.sync.dma_start(
        out=AP(X.tensor, (WS + 5) * C, [[XF, 128], [1, 5 * C]]),
        in_=AP(x_d, (PW + 5) * C, [[4 * PW * C, 16], [8 * C, 8], [1, 5 * C]]),
    )
    for s in (2, 0, 3, 4, 5):
        nc.sync.dma_start(
            out=AP(X.tensor, s * WS * C, [[XF, 128], [1, WS * C]]),
            in_=AP(x_d, s * PW * C,
                   [[4 * PW * C, 16], [8 * C, 8], [1, WS * C]]),
        )

    # per hl-row grp: transposes -> xcm, t2 matmuls, W9 scatter.
    # grp0 runs at 4-tile granularity to shorten the path to the first
    # products; later grps use full-8 batches.
    for grp in range(4):
        nsub = 2 if grp == 0 else 1
        for sub in range(nsub):
            tts = range(sub * 8 // nsub, (sub + 1) * 8 // nsub)
            for half in range(2):
                pt = pst.tile([128, 1024 // nsub], bf16, padded_shape=[128, 1024])
                for i, tt in enumerate(tts):
                    slot = (grp + 1) * WS + (tt + 1)
                    nc.tensor.transpose(
                        out=pt[:, i * 128:(i + 1) * 128],
                        in_=AP(X.tensor, slot * C + half * 128,
                               [[XF, 128], [1, 128]]),
                        identity=ident[:],
                    )
                cpeng = nc.vector.tensor_copy if grp < 2 else (
                    lambda out, in_: nc.scalar.copy(out=out, in_=in_))
                cpeng(
                    out=xcm[:, (half * 32 + grp * 8 + tts[0]) * 128:
                            (half * 32 + grp * 8 + tts[-1] + 1) * 128],
                    in_=pt[:],
                )
            for tt in tts:
                t = grp * 8 + tt
                ps2 = ps2p.tile([128, M144], f32)
                xc0 = AP(xcm.tensor, t * 128, [[2 * 32 * 128, 128], [1, 128]])
                xc1 = AP(xcm.tensor, (32 + t) * 128, [[2 * 32 * 128, 128], [1, 128]])
                nc.tensor.matmul(ps2[:], lhsT=xc0, rhs=wc0[:], start=True, stop=False)
                nc.tensor.matmul(ps2[:], lhsT=xc1, rhs=wc1[:], start=False,
                                 stop=not with_bias)
                if with_bias:
                    nc.tensor.matmul(ps2[:], lhsT=ones1[:], rhs=bcsb[:],
                                     start=False, stop=True)
                nc.scalar.copy(
                    out=AP(W9c[grp].tensor, tt * M144, [[W9F, 128], [1, M144]]),
                    in_=ps2[:],
                )

    # products (DVE+Pool, g-ascending) -> M9; fold (PE) per g-quarter;
    # evacuate (Act); store.  Groups are 9-strided in f (f = 9*(16g+gc)),
    # so a fold block = (all 8 wl) x (64 groups) only needs pieces for
    # g in [4q, 4q+4) -- the fold trails the g-ordered products closely.
    for c in range(NCHUNK):
        M9 = m9p.tile([128, M9F], bf16, tag="m9")
        for (g, gc0, ngc, j0, nj, ki) in PIECES:
            di, dj = ki // 3 - 1, ki % 3 - 1
            f0 = 144 * g + 9 * gc0 + j0
            in0 = AP(X.tensor,
                     ((c + di + 1) * WS + dj + 1) * C + f0 - 256 * ki,
                     [[XF, 128], [256, WLC], [9, ngc], [1, nj]])
            in1 = AP(W9c[c].tensor, 9 * g + j0,
                     [[W9F, 128], [M144, WLC], [0, ngc], [1, nj]])
            o = AP(M9.tensor, f0,
                   [[M9F, 128], [2304, WLC], [9, ngc], [1, nj]])
            kis = pool_kis if c < NCHUNK - 1 else (1, 4, 7)
            eng = nc.gpsimd if (ki in kis or nj < 9) else nc.vector
            eng.tensor_mul(o, in0, in1)
        outc = outp.tile([128, WLC * 256], bf16, tag="outc")
        last = c == NCHUNK - 1
        fold_blocks = ([(0, 64), (64, 64), (128, 64), (192, 64)] if not last
                       else [(0, 64), (64, 64), (128, 64), (192, 32), (224, 32)])
        for bi, (g0, ng) in enumerate(fold_blocks):
            ps = foldp.tile([128, 512], f32, padded_shape=[128, 512])
            for j in range(9):
                nc.tensor.matmul(
                    ps[:, :WLC * ng // 8 * 8][:, :ng * WLC],
                    lhsT=ident[:],
                    rhs=AP(M9.tensor, 9 * g0 + j,
                           [[M9F, 128], [2304, WLC], [9, ng]]),
                    start=(j == 0),
                    stop=(j == 8),
                )
            odst = AP(outc.tensor, g0,
                      [[WLC * 256, 128], [256, WLC], [1, ng]])
            psrc = AP(ps.tensor, 0, [[512, 128], [1, ng * WLC]])
            if last and bi == len(fold_blocks) - 1:
                nc.vector.tensor_copy(out=odst, in_=psrc)
            else:
                nc.scalar.copy(out=odst, in_=psrc)
        # store hl-row c in partition-major layout; host un-permutes.
        # Last chunk: store first 3 blocks early, tail block separately.
        if last:
            nc.sync.dma_start(
                out=AP(out_d, c * WLC * 256, [[32 * C, 128], [1, 3 * 512]]),
                in_=AP(outc.tensor, 0, [[WLC * 256, 128], [1, 3 * 512]]),
            )
            nc.sync.dma_start(
                out=AP(out_d, c * WLC * 256 + 3 * 512, [[32 * C, 128], [1, 512]]),
                in_=AP(outc.tensor, 3 * 512, [[WLC * 256, 128], [1, 512]]),
            )
        else:
            nc.sync.dma_start(
                out=AP(out_d, c * WLC * 256, [[32 * C, 128], [1, WLC * 256]]),
                in_=AP(outc.tensor, 0, [[WLC * 256, 128], [1, WLC * 256]]),
            )


def _get_program(n_repeat=1, with_bias=True, pool_kis=(1, 7)):
    key = ("nc", n_repeat, with_bias, tuple(pool_kis))
    if key not in _cache:
        _cache[key] = _build_program(n_repeat, with_bias, pool_kis)
    return _cache[key]


def _make_in_maps(inputs):
    x, W1, b1, W2, b2 = (inputs[k] for k in ("x", "W1", "b1", "W2", "b2"))
    bf = ml_dtypes.bfloat16
    Wc = (np.asarray(W1, np.float32) @ np.asarray(W2, np.float32))
    bc = (np.asarray(b1, np.float32) @ np.asarray(W2, np.float32)
          + np.asarray(b2, np.float32))
    wc_h = np.ascontiguousarray(Wc.astype(bf).reshape(2, 128, M144))
    bc_h = np.ascontiguousarray(bc.astype(bf).reshape(1, M144))
    xp = np.zeros((NCORES, PW, PW, C), dtype=bf)
    xp[:, 1:65, 1:65, :] = np.asarray(x).astype(bf)
    return [
        {
            "xpad": np.ascontiguousarray(xp[i]),
            "wc": wc_h,
            "bc": bc_h,
        }
        for i in range(NCORES)
    ]


def _unpermute(raw):
    # [128, 32*256] p-major -> [64, 64, 256]: p = hb*8+wb, free = (hl, wl, co)
    return (np.asarray(raw).reshape(16, 8, 4, 8, 256)
            .transpose(0, 2, 1, 3, 4)
            .reshape(64, 64, 256))


def kernel(x, W1, b1, W2, b2, trace=False):
    from concourse.bass_utils import run_bass_kernel_spmd

    bc = np.asarray(b1, np.float32) @ np.asarray(W2, np.float32) + np.asarray(
        b2, np.float32)
    nc = _get_program(with_bias=bool(np.any(bc != 0.0)))
    in_maps = _make_in_maps(dict(x=x, W1=W1, b1=b1, W2=W2, b2=b2))
    res = run_bass_kernel_spmd(nc, in_maps, core_ids=list(range(NCORES)),
                               trace=trace)
    out = np.stack([_unpermute(res.results[i]["out"]) for i in range(NCORES)],
                   axis=0).astype(np.float32)
    if trace:
        return out, res
    return out


# revision 14
# speedup vs baseline: 1.4738x; 1.0346x over previous
# Trainium2 Bass kernel for nn_Invo2D, v5 (p-major store, host unpermute): 2D-tiled partitions, padded-x
# single load, bf16 output (host upcast).
#
#   t2 = x @ Wc + bc     (Wc = W1@W2, bc = b1@W2+b2 host-folded, bf16)
#   out[pix, co] = sum_j t2[pix, 9*(co//16)+j] * x_tap(f)[pix, ch(f)], f = 9*co+j
#
# Per core (1 image): partition p = hb*8 + wb (16 h-blocks x 8 w-blocks);
# each partition owns a 4x8 pixel block with a 1-pixel 2D halo: free dim =
# (6 h-slots x 10 w-slots x 256 ch).  x is zero-padded to [66,66,256] bf16
# on the host so the halo loads are a single in-bounds DMA per h-slice.
# Both spatial taps are free-dim offsets -> ONE x buffer, no shifted copies.
# Products: DVE 2x piece-muls into 9-packed M9 (2 rotating hl-chunk bufs).
# Fold over 9 taps: TensorE identity-lhsT PSUM-accumulate; Act evacuates
# PSUM -> SBUF bf16; DMA stores bf16; host upcasts to f32.
import numpy as np
import ml_dtypes

H, W, C = 64, 64, 256
M144 = 144
NCORES = 8
HS, WS = 6, 10          # h-slots, w-slots per partition (4+2, 8+2)
XF = HS * WS * C        # 15360 bf16 per partition
WLC = 8                 # positions per chunk = one hl row
NCHUNK = 4
M9F = WLC * 2304
W9F = WLC * M144
PW = 66                 # padded row length

_cache = {}


def _rect_decomp(r0, r1):
    out = []
    gc0, j0 = divmod(r0, 9)
    if j0 != 0:
        end = min(r1, (gc0 + 1) * 9)
        out.append((gc0, 1, j0, end - r0))
        r0 = end
        if r0 == r1:
            return out
        gc0, j0 = divmod(r0, 9)
    nfull = (r1 - r0) // 9
    if nfull:
        out.append((gc0, nfull, 0, 9))
        r0 += nfull * 9
        gc0 += nfull
    if r0 < r1:
        out.append((gc0, 1, 0, r1 - r0))
    return out


def _build_pieces():
    pieces = []
    for g in range(16):
        f_lo, f_hi = 144 * g, 144 * g + 144
        cuts = [f_lo] + [256 * k for k in range(1, 9) if f_lo < 256 * k < f_hi] + [f_hi]
        for a, b in zip(cuts, cuts[1:]):
            ki = a // 256
            for (gc0, ngc, j0, nj) in _rect_decomp(a - 144 * g, b - 144 * g):
                pieces.append((g, gc0, ngc, j0, nj, ki))
    return pieces


def _build_program(n_repeat=1, with_bias=True, pool_kis=()):
    import concourse.bass as bass
    import concourse.tile as tile
    from concourse import bacc, mybir
    from concourse.masks import make_identity

    f32 = mybir.dt.float32
    bf16 = mybir.dt.bfloat16
    AP = bass.AP

    nc = bacc.Bacc(None, target_bir_lowering=False)
    x_d = nc.dram_tensor("xpad", [PW, PW, C], bf16, kind="ExternalInput")
    wc_d = nc.dram_tensor("wc", [2, 128, M144], bf16, kind="ExternalInput")
    bc_d = nc.dram_tensor("bc", [1, M144], bf16, kind="ExternalInput")
    out_d = nc.dram_tensor("out", [128, 32 * C], bf16, kind="ExternalOutput")

    PIECES = _build_pieces()

    with tile.TileContext(nc) as tc:
        with (
            tc.tile_pool(name="singles", bufs=1) as singles,
            tc.tile_pool(name="xbufs", bufs=1) as xbufs,
            tc.tile_pool(name="w9p", bufs=1) as w9p,
            tc.tile_pool(name="m9p", bufs=2) as m9p,
            tc.tile_pool(name="outp", bufs=2) as outp,
            tc.tile_pool(name="pst", bufs=2, space="PSUM") as pst,
            tc.tile_pool(name="ps2p", bufs=3, space="PSUM") as ps2p,
            tc.tile_pool(name="foldp", bufs=3, space="PSUM") as foldp,
        ):
            ident = singles.tile([128, 128], bf16)
            make_identity(nc, ident[:])
            wc0 = singles.tile([128, M144], bf16)
            wc1 = singles.tile([128, M144], bf16)
            nc.scalar.dma_start(out=wc0[:], in_=wc_d[0])
            nc.scalar.dma_start(out=wc1[:], in_=wc_d[1])
            bcsb = singles.tile([1, M144], bf16)
            ones1 = singles.tile([1, 128], bf16)
            if with_bias:
                nc.scalar.dma_start(out=bcsb[:], in_=bc_d[:])
                nc.vector.memset(ones1[:], 1.0)
            xcm = singles.tile([128, 2 * 32 * 128], bf16)
            W9c = [w9p.tile([128, W9F], bf16, name=f"w9_{c}", tag=f"w9_{c}")
                   for c in range(NCHUNK)]

            for _rep in range(n_repeat):
                _image_body(nc, bass, mybir,
                            xbufs, m9p, outp, pst, ps2p, foldp,
                            ident, wc0, wc1, bcsb, ones1, xcm, W9c,
                            x_d, out_d, PIECES, with_bias, pool_kis)
    nc.compile()
    return nc


def _image_body(nc, bass, mybir,
                xbufs, m9p, outp, pst, ps2p, foldp,
                ident, wc0, wc1, bcsb, ones1, xcm, W9c,
                x_d, out_d, PIECES, with_bias, pool_kis=()):
    f32 = mybir.dt.float32
    bf16 = mybir.dt.bfloat16
    AP = bass.AP

    X = xbufs.tile([128, XF], bf16, tag="X")
    # h-slice loads: slot row s (0..5) <- xpad row hb*4 + s, cols wb*8..+10.
    # partition p = hb*8 + wb.  Order: s=1 gates grp0's transposes, then s=2
    # (grp1), s=0 (chunk0 products), then the rest.
    nc.sync.dma_start(
        out=AP(X.tensor, WS * C, [[XF, 128], [1, 5 * C]]),
        in_=AP(x_d, PW * C, [[4 * PW * C, 16], [8 * C, 8], [1, 5 * C]]),
    )
    nc.sync.dma_start(
        out=AP(X.tensor, (WS + 5) * C, [[XF, 128], [1, 5 * C]]),
        in_=AP(x_d, (PW + 5) * C, [[4 * PW * C, 16], [8 * C, 8], [1, 5 * C]]),
    )
    for s in (2, 0, 3, 4, 5):
        nc.sync.dma_start(
            out=AP(X.tensor, s * WS * C, [[XF, 128], [1, WS * C]]),
            in_=AP(x_d, s * PW * C,
                   [[4 * PW * C, 16], [8 * C, 8], [1, WS * C]]),
        )

    # per hl-row grp: transposes -> xcm, t2 matmuls, W9 scatter.
    # grp0 runs at 4-tile granularity to shorten the path to the first
    # products; later grps use full-8 batches.
    for grp in range(4):
        nsub = 2 if grp == 0 else 1
        for sub in range(nsub):
            tts = range(sub * 8 // nsub, (sub + 1) * 8 // nsub)
            for half in range(2):
                pt = pst.tile([128, 1024 // nsub], bf16, padded_shape=[128, 1024])
                for i, tt in enumerate(tts):
                    slot = (grp + 1) * WS + (tt + 1)
                    nc.tensor.transpose(
                        out=pt[:, i * 128:(i + 1) * 128],
                        in_=AP(X.tensor, slot * C + half * 128,
                               [[XF, 128], [1, 128]]),
                        identity=ident[:],
                    )
                cpeng = nc.vector.tensor_copy if grp < 2 else (
                    lambda out, in_: nc.scalar.copy(out=out, in_=in_))
                cpeng(
                    out=xcm[:, (half * 32 + grp * 8 + tts[0]) * 128:
                            (half * 32 + grp * 8 + tts[-1] + 1) * 128],
                    in_=pt[:],
                )
            for tt in tts:
                t = grp * 8 + tt
                ps2 = ps2p.tile([128, M144], f32)
                xc0 = AP(xcm.tensor, t * 128, [[2 * 32 * 128, 128], [1, 128]])
                xc1 = AP(xcm.tensor, (32 + t) * 128, [[2 * 32 * 128, 128], [1, 128]])
                nc.tensor.matmul(ps2[:], lhsT=xc0, rhs=wc0[:], start=True, stop=False)
                nc.tensor.matmul(ps2[:], lhsT=xc1, rhs=wc1[:], start=False,
                                 stop=not with_bias)
                if with_bias:
                    nc.tensor.matmul(ps2[:], lhsT=ones1[:], rhs=bcsb[:],
                                     start=False, stop=True)
                nc.scalar.copy(
                    out=AP(W9c[grp].tensor, tt * M144, [[W9F, 128], [1, M144]]),
                    in_=ps2[:],
                )

    # products (DVE+Pool, g-ascending) -> M9; fold (PE) per g-quarter;
    # evacuate (Act); store.  Groups are 9-strided in f (f = 9*(16g+gc)),
    # so a fold block = (all 8 wl) x (64 groups) only needs pieces for
    # g in [4q, 4q+4) -- the fold trails the g-ordered products closely.
    for c in range(NCHUNK):
        M9 = m9p.tile([128, M9F], bf16, tag="m9")
        for (g, gc0, ngc, j0, nj, ki) in PIECES:
            di, dj = ki // 3 - 1, ki % 3 - 1
            f0 = 144 * g + 9 * gc0 + j0
            in0 = AP(X.tensor,
                     ((c + di + 1) * WS + dj + 1) * C + f0 - 256 * ki,
                     [[XF, 128], [256, WLC], [9, ngc], [1, nj]])
            in1 = AP(W9c[c].tensor, 9 * g + j0,
                     [[W9F, 128], [M144, WLC], [0, ngc], [1, nj]])
            o = AP(M9.tensor, f0,
                   [[M9F, 128], [2304, WLC], [9, ngc], [1, nj]])
            kis = pool_kis if c < NCHUNK - 1 else (1, 4, 7)
            eng = nc.gpsimd if (ki in kis or nj < 9) else nc.vector
            eng.tensor_mul(o, in0, in1)
        outc = outp.tile([128, WLC * 256], bf16, tag="outc")
        last = c == NCHUNK - 1
        # last chunk: PE folds only groups 0..191; the DVE folds groups
        # 192..255 itself (bf16 add-tree) after its products, so PE's queue
        # tail and the final PSUM evacuation drop out of the end chain.
        fold_blocks = ([(0, 64), (64, 64), (128, 64), (192, 64)] if not last
                       else [(0, 64), (64, 64), (128, 64)])
        for bi, (g0, ng) in enumerate(fold_blocks):
            ps = foldp.tile([128, 512], f32, padded_shape=[128, 512])
            for j in range(9):
                nc.tensor.matmul(
                    ps[:, :WLC * ng // 8 * 8][:, :ng * WLC],
                    lhsT=ident[:],
                    rhs=AP(M9.tensor, 9 * g0 + j,
                           [[M9F, 128], [2304, WLC], [9, ng]]),
                    start=(j == 0),
                    stop=(j == 8),
                )
            odst = AP(outc.tensor, g0,
                      [[WLC * 256, 128], [256, WLC], [1, ng]])
            psrc = AP(ps.tensor, 0, [[512, 128], [1, ng * WLC]])
            nc.scalar.copy(out=odst, in_=psrc)
        if last:
            # DVE add-tree over j for groups 192..255, all 8 wl positions.
            # M9 addr for (wl, 192+gi, j) = wl*2304 + 1728 + 9*gi + j.
            T1 = outp.tile([128, 2048], bf16, tag="tree1")
            T2 = outp.tile([128, 1024], bf16, tag="tree2")
            T3 = outp.tile([128, 512], bf16, tag="tree3")
            nc.vector.tensor_add(
                AP(T1.tensor, 0, [[2048, 128], [256, WLC], [4, 64], [1, 4]]),
                AP(M9.tensor, 1728, [[M9F, 128], [2304, WLC], [9, 64], [1, 4]]),
                AP(M9.tensor, 1732, [[M9F, 128], [2304, WLC], [9, 64], [1, 4]]),
            )
            nc.vector.tensor_add(
                AP(T2.tensor, 0, [[1024, 128], [128, WLC], [2, 64], [1, 2]]),
                AP(T1.tensor, 0, [[2048, 128], [256, WLC], [4, 64], [1, 2]]),
                AP(T1.tensor, 2, [[2048, 128], [256, WLC], [4, 64], [1, 2]]),
            )
            nc.vector.tensor_add(
                AP(T3.tensor, 0, [[512, 128], [64, WLC], [1, 64]]),
                AP(T2.tensor, 0, [[1024, 128], [128, WLC], [2, 64]]),
                AP(T2.tensor, 1, [[1024, 128], [128, WLC], [2, 64]]),
            )
            nc.vector.tensor_add(
                AP(outc.tensor, 192, [[WLC * 256, 128], [256, WLC], [1, 64]]),
                AP(T3.tensor, 0, [[512, 128], [64, WLC], [1, 64]]),
                AP(M9.tensor, 1736, [[M9F, 128], [2304, WLC], [9, 64]]),
            )
        # store hl-row c in partition-major layout; host un-permutes.
        # Last chunk: store first 3 blocks early, tail block separately.
        if last:
            nc.sync.dma_start(
                out=AP(out_d, c * WLC * 256, [[32 * C, 128], [1, 3 * 512]]),
                in_=AP(outc.tensor, 0, [[WLC * 256, 128], [1, 3 * 512]]),
            )
            nc.sync.dma_start(
                out=AP(out_d, c * WLC * 256 + 3 * 512, [[32 * C, 128], [1, 512]]),
                in_=AP(outc.tensor, 3 * 512, [[WLC * 256, 128], [1, 512]]),
            )
        else:
            nc.sync.dma_start(
                out=AP(out_d, c * WLC * 256, [[32 * C, 128], [1, WLC * 256]]),
                in_=AP(outc.tensor, 0, [[WLC * 256, 128], [1, WLC * 256]]),
            )


def _get_program(n_repeat=1, with_bias=True, pool_kis=(1, 7)):
    key = ("nc", n_repeat, with_bias, tuple(pool_kis))
    if key not in _cache:
        _cache[key] = _build_program(n_repeat, with_bias, pool_kis)
    return _cache[key]


def _make_in_maps(inputs):
    x, W1, b1, W2, b2 = (inputs[k] for k in ("x", "W1", "b1", "W2", "b2"))
    bf = ml_dtypes.bfloat16
    Wc = (np.asarray(W1, np.float32) @ np.asarray(W2, np.float32))
    bc = (np.asarray(b1, np.float32) @ np.asarray(W2, np.float32)
          + np.asarray(b2, np.float32))
    wc_h = np.ascontiguousarray(Wc.astype(bf).reshape(2, 128, M144))
    bc_h = np.ascontiguousarray(bc.astype(bf).reshape(1, M144))
    xp = np.zeros((NCORES, PW, PW, C), dtype=bf)
    xp[:, 1:65, 1:65, :] = np.asarray(x).astype(bf)
    return [
        {
            "xpad": np.ascontiguousarray(xp[i]),
            "wc": wc_h,
            "bc": bc_h,
        }
        for i in range(NCORES)
    ]


def _unpermute(raw):
    # [128, 32*256] p-major -> [64, 64, 256]: p = hb*8+wb, free = (hl, wl, co)
    return (np.asarray(raw).reshape(16, 8, 4, 8, 256)
            .transpose(0, 2, 1, 3, 4)
            .reshape(64, 64, 256))


def kernel(x, W1, b1, W2, b2, trace=False):
    from concourse.bass_utils import run_bass_kernel_spmd

    bc = np.asarray(b1, np.float32) @ np.asarray(W2, np.float32) + np.asarray(
        b2, np.float32)
    nc = _get_program(with_bias=bool(np.any(bc != 0.0)))
    in_maps = _make_in_maps(dict(x=x, W1=W1, b1=b1, W2=W2, b2=b2))
    res = run_bass_kernel_spmd(nc, in_maps, core_ids=list(range(NCORES)),
                               trace=trace)
    out = np.stack([_unpermute(res.results[i]["out"]) for i in range(NCORES)],
                   axis=0).astype(np.float32)
    if trace:
        return out, res
    return out
